# revision 37
# baseline (speedup 1.0000x reference)
"""DGCN layer on 8 TRN2 NeuronCores (Bass/Tile) — v2.

Differences from v1 (19.5ms -> target ~8ms):
  - Dense GEMMs commuted past the SpMMs: spmm(adj, x@W) == spmm(adj, x)@W,
    so gathers read the raw/pre-GEMM node tables and the [F,F] GEMM runs
    per 128-row output tile in the epilogue. This kills v1's "stage 0"
    (redundant full-table s1/s2 dense precompute, ~2ms of HBM traffic).
  - The two uv-adjacency SpMMs (B: item_ho from vfea, C: user_ho2 from
    user_ho) share one gather pass over a column-interleaved table
    [vfea | user_ho] with 512B descriptors — per-descriptor cost is fixed
    (~20ns/desc measured for 256B and 512B alike), so this removes one of
    four full gather sets AND halves their DVE scatter-matrix builds
    (B and C share adjacency values => identical S.T matrices).
  - AllGathers are chunked and overlap the producing stage's tail.
  - Edges are column-sorted within each (tile, block) bucket for better
    HBM locality of the random gathers.

Stages:  A (vu, ufea)          -> user_ho rows  -> tbc_sh[:,128:256]
         AG(tbc_sh)            -> tbc_full [N,256] = [vfea | user_ho]
         BC (uv, tbc_full):    B-half -> item_ho rows -> td_sh
                               C-half -> @W3+b3,prelu -> union -> user_out
         AG(td_sh)             -> td_full [N,128]
         D (vu, td_full)       -> @W4+b4,prelu -> union -> item_out
"""

import numpy as np
import ml_dtypes
from contextlib import ExitStack

BF16 = ml_dtypes.bfloat16

NC = 8
F = 128
ALPHA = 0.2

N = 100000
RPC = N // NC
NBLK = 4
BLK = N // NBLK
NT = (RPC + 127) // 128
TAIL = RPC - 128 * (NT - 1)

ST_A = 4                  # supertile row-tiles for vu stages (256B gathers)
ST_BC = 2                 # for the paired uv stage (512B gathers)
AG_PIECES = 10            # AllGather chunks overlapped with producer stage
RPP = RPC // AG_PIECES    # shard rows per AG piece


def _perm(g):
    """Full-table row layout is piece-major (piece, core, row) so each AG
    piece writes a contiguous range. Maps node id -> table row."""
    g = np.asarray(g, np.int64)
    c, r = g // RPC, g % RPC
    return (r // RPP) * (NC * RPP) + c * RPP + (r % RPP)

LAST_EXEC_NS = None


def _set_dims(n):
    global N, RPC, BLK, NT, TAIL, RPP
    N = n
    RPC = N // NC
    BLK = N // NBLK
    NT = (RPC + 127) // 128
    TAIL = RPC - 128 * (NT - 1)
    RPP = RPC // AG_PIECES


# ---------------------------------------------------------------- host prep

class AdjLayout:
    """Static (core-invariant) instruction-stream layout for one adjacency.
    C: per-tile chunk cap (max over cores of that tile's bucket ceil)."""

    def __init__(self, C, st_t):
        self.st_t = st_t
        self.C = list(C)
        self.sts = [list(range(i, min(i + st_t, NT)))
                    for i in range(0, NT, st_t)]
        self.pstart = np.zeros((NT, NBLK), np.int64)
        self.calls = []   # (st_i, b, edge_start, num_idxs)
        off = 0
        for si, stt in enumerate(self.sts):
            for b in range(NBLK):
                cs = off
                for t in stt:
                    self.pstart[t, b] = off
                    off += self.C[t] * 128
                self.calls.append((si, b, cs, off - cs))
        self.ntot = off
        self.nch = off // 128

    def chunk_col(self, t, b, k):
        return int(self.pstart[t, b]) // 128 + k

    def rank(self, st_edge_start, t, b, k):
        return (int(self.pstart[t, b]) - st_edge_start) // 128 + k


def _prep_adj(rows, cols, vals, st_t):
    """rows sorted. Returns (layout, per-core dict of idx/rr/vv arrays).
    Edges are col-sorted within each (tile, block) bucket."""
    rows = np.asarray(rows, np.int64)
    cols = np.asarray(cols, np.int64)
    vals = np.asarray(vals, np.float32)
    bounds = np.searchsorted(rows, np.arange(NC + 1) * RPC, side="left")

    per_core = []
    tcols = _perm(cols)      # table rows are piece-major permuted
    cmax = np.zeros(NT, np.int64)
    for c in range(NC):
        sl = slice(bounds[c], bounds[c + 1])
        r_loc = rows[sl] - c * RPC
        t_id = r_loc >> 7
        b_id = tcols[sl] // BLK
        key = (t_id * NBLK + b_id).astype(np.int64)
        cnt = np.bincount(key, minlength=NT * NBLK).reshape(NT, NBLK)
        per_core.append((sl, r_loc, t_id, b_id, key, cnt))
        cmax = np.maximum(cmax, cnt.max(axis=1))
    L = AdjLayout(np.maximum((cmax + 127) // 128, 1), st_t)

    cores = []
    for c in range(NC):
        sl, r_loc, t_id, b_id, key, cnt = per_core[c]
        n = r_loc.shape[0]
        order = np.lexsort((tcols[sl], key))   # bucket-major, col-sorted
        ks = key[order]
        starts = np.zeros(NT * NBLK + 1, np.int64)
        np.cumsum(cnt.reshape(-1), out=starts[1:])
        within = np.arange(n, dtype=np.int64) - starts[ks]
        dest = L.pstart.reshape(-1)[ks] + within

        idx_flat = np.zeros(L.ntot, np.int16)
        rr_flat = np.zeros(L.ntot, np.float32)
        vv_flat = np.zeros(L.ntot, np.float32)
        idx_flat[dest] = (tcols[sl][order] - b_id[order] * BLK).astype(np.int16)
        rr_flat[dest] = (r_loc[order] - (t_id[order] << 7)).astype(np.float32)
        vv_flat[dest] = vals[sl][order]

        idx16 = np.ascontiguousarray(
            np.tile(idx_flat.reshape(-1, 16).T, (8, 1)))      # [128, ntot/16]
        rr = np.ascontiguousarray(rr_flat.reshape(-1, 128).T)  # [128, nch]
        vv = np.ascontiguousarray(vv_flat.reshape(-1, 128).T)
        cores.append(dict(idx=idx16, rr=rr, vv=vv))
    return L, cores


# ------------------------------------------------------------- bass program

def _build(LVU, LUV, qmap=None):
    import concourse.bacc as bacc
    import concourse.mybir as mybir
    from concourse.tile import TileContext

    dt = mybir.dt
    AOT = mybir.AluOpType
    ACTF = mybir.ActivationFunctionType

    nc = bacc.Bacc("TRN2", num_devices=NC, num_swdge_queues=4)

    def din(name, shape, dty):
        return nc.dram_tensor(name, shape, dty, kind="ExternalInput")

    ufea_rows = din("ufea_rows", [N, F], dt.bfloat16)      # A's gather table
    vfea_sh = din("vfea_sh", [RPC, F], dt.bfloat16)        # BC prefill half
    feaT_u = din("feaT_u", [F, RPC], dt.bfloat16)
    feaT_v = din("feaT_v", [F, RPC], dt.bfloat16)
    wts = {k: din(k, [F, F], dt.bfloat16)
           for k in ["W1", "W2", "W3", "W4", "Wut", "Wub", "Wit", "Wib"]}
    brows = {k: din(k, [1, F], dt.bfloat16)
             for k in ["b1_row", "b2_row", "bu_row", "bi_row"]}
    bcols = {k: din(k, [F, 1], dt.float32) for k in ["b3", "b4", "zcol"]}
    iota_in = din("iota", [128, 128], dt.bfloat16)
    ones_in = din("ones", [1, 128], dt.bfloat16)

    meta = {}
    for tag, L in [("vu", LVU), ("uv", LUV)]:
        meta[tag] = dict(
            idx=din(f"idx_{tag}", [128, L.ntot // 16], dt.int16),
            rr=din(f"rr_{tag}", [128, L.nch], dt.float32),
            vv=din(f"vv_{tag}", [128, L.nch], dt.float32),
            L=L,
        )

    out_user = nc.dram_tensor("user_out", [RPC, F], dt.float32,
                              kind="ExternalOutput")
    out_item = nc.dram_tensor("item_out", [RPC, F], dt.float32,
                              kind="ExternalOutput")

    tbc_sh = nc.dram_tensor("tbc_sh", [RPC, 2 * F], dt.bfloat16,
                            kind="Internal")
    tbc_full = nc.dram_tensor("tbc_full", [N, 2 * F], dt.bfloat16,
                              kind="Internal", addr_space="Shared")
    td_sh = nc.dram_tensor("td_sh", [RPC, F], dt.bfloat16, kind="Internal")
    td_full = nc.dram_tensor("td_full", [N, F], dt.bfloat16,
                             kind="Internal", addr_space="Shared")

    gather_insts = []

    with TileContext(nc) as tc, ExitStack() as ctx:
        consts = ctx.enter_context(tc.tile_pool(name="consts", bufs=1))
        metap = ctx.enter_context(tc.tile_pool(name="meta", bufs=1))
        idxp = ctx.enter_context(tc.tile_pool(name="idx", bufs=3))
        gp = ctx.enter_context(tc.tile_pool(
            name="gather", bufs=int(__import__("os").environ.get("KB_GB", "2"))))
        sp = ctx.enter_context(tc.tile_pool(name="sT", bufs=8))
        hp = ctx.enter_context(tc.tile_pool(name="hidden", bufs=4))
        ob = ctx.enter_context(tc.tile_pool(name="outsb", bufs=3))
        fq = ctx.enter_context(tc.tile_pool(name="feach", bufs=3))
        psA = ctx.enter_context(tc.tile_pool(name="psA", bufs=6, space="PSUM"))
        ps2 = ctx.enter_context(tc.tile_pool(name="ps2", bufs=1, space="PSUM"))

        def cload(handle, shape, dty, tag):
            t = consts.tile(shape, dty, tag=tag)
            nc.sync.dma_start(t[:], handle[:])
            return t

        wt = {k: cload(v, [F, F], dt.bfloat16, f"c_{k}")
              for k, v in wts.items()}
        brt = {k: cload(v, [1, F], dt.bfloat16, f"c_{k}")
               for k, v in brows.items()}
        bct = {k: cload(v, [F, 1], dt.float32, f"c_{k}")
               for k, v in bcols.items()}
        iota = cload(iota_in, [128, 128], dt.bfloat16, "c_iota")
        ones = cload(ones_in, [1, 128], dt.bfloat16, "c_ones")

        def rows_of(t):
            return 128 if t < NT - 1 else TAIL

        # ---- prefill tbc_sh[:, 0:F] with vfea rows (overlaps stage A)
        for t in range(NT):
            R = rows_of(t)
            vtile = fq.tile([128, F], dt.bfloat16, tag="pf")
            nc.sync.dma_start(vtile[:R, :], vfea_sh[t * 128:t * 128 + R, :])
            nc.sync.dma_start(tbc_sh[t * 128:t * 128 + R, 0:F], vtile[:R, :])

        # ---- chunked AllGather helpers: piece p = shard rows
        # [p*RPP, (p+1)*RPP) -> full rows [p*NC*RPP + NC*r0 ...] contiguous
        def ag_piece(sh, full, width, p):
            r0, r1 = p * RPP, (p + 1) * RPP
            nc.gpsimd.collective_compute(
                "AllGather", mybir.AluOpType.bypass,
                replica_groups=[list(range(NC))],
                ins=[sh[r0:r1, :]], outs=[full[NC * r0:NC * r1, :]],
            )

        def ag_plan(L):
            """{si: [piece]}: piece p fires one supertile AFTER the one whose
            tiles finish shard row (p+1)*RPP - 1, so the collective (a Pool
            instruction) never stalls Pool waiting on epilogue stores."""
            tile_to_si = {}
            for si, stt in enumerate(L.sts):
                for t in stt:
                    tile_to_si[t] = si
            plan = {}
            for p in range(AG_PIECES):
                last_tile = min(((p + 1) * RPP - 1) // 128, NT - 1)
                si = min(tile_to_si[last_tile] + 1, len(L.sts) - 1)
                plan.setdefault(si, []).append(p)
            return plan

        # ---------------- generic spmm stage
        def spmm(adj, table_ap, esz, epis, agp=None, inject=None):
            """table_ap: block index b -> [BLK, esz] AP. epis: list of
            (col_lo, epi_fn); one PSUM accumulator per epi. agp: {si:
            [(r0,r1)], "fire": fn} AG pieces fired after supertile si."""
            m = meta[adj]
            L = m["L"]
            rr = metap.tile([128, L.nch], dt.float32, tag="rr")
            nc.sync.dma_start(rr[:], m["rr"][:])
            vv = metap.tile([128, L.nch], dt.float32, tag="vv")
            nc.sync.dma_start(vv[:], m["vv"][:])

            for si, stt in enumerate(L.sts):
                if si == 1 and inject is not None:
                    inject()
                st_cs = L.calls[si * NBLK][2]
                st_n = sum(L.calls[si * NBLK + b][3] for b in range(NBLK))
                it = idxp.tile([128, st_n // 16], dt.int16, tag="idx")
                nc.sync.dma_start(
                    it[:], m["idx"][:, st_cs // 16: (st_cs + st_n) // 16])
                import os as _os
                gsplit = int(_os.environ.get("KB_GSPLIT", "4"))
                gbufs = []
                cstarts = []
                for b in range(NBLK):
                    _, _, cs, nidx = L.calls[si * NBLK + b]
                    g = gp.tile([128, nidx // 128, esz], dt.bfloat16,
                                tag=f"g{b}")
                    co = (cs - st_cs) // 16
                    nparts = gsplit if nidx // 128 >= gsplit else 1
                    bnd = [(nidx // 128) * p // nparts * 128
                           for p in range(nparts + 1)]
                    for p in range(nparts):
                        n0, n1 = bnd[p], bnd[p + 1]
                        if n1 == n0:
                            continue
                        ordinal = len(gather_insts)
                        qn = qmap[ordinal] if qmap else ordinal % 4
                        inst = nc.gpsimd.dma_gather(
                            g[:, n0 // 128:n1 // 128, :], table_ap(b),
                            it[:, co + n0 // 16:co + n1 // 16],
                            n1 - n0, n1 - n0, esz,
                            single_packet=False, queue_num=qn)
                        gather_insts.append(inst)
                    gbufs.append(g)
                    cstarts.append(cs)

                for t in stt:
                    psTs = [psA.tile([128, 128], dt.float32, tag="psA",
                                     name=f"psT{e}_{t}")
                            for e in range(len(epis))]
                    nchunks = NBLK * L.C[t]
                    ji = 0
                    for b in range(NBLK):
                        for k in range(L.C[t]):
                            j = L.chunk_col(t, b, k)
                            rk = L.rank(cstarts[b], t, b, k)
                            sT = sp.tile([128, 128], dt.bfloat16, tag="sT")
                            nc.vector.tensor_scalar(
                                sT[:], iota[:], rr[:, j:j + 1],
                                vv[:, j:j + 1], AOT.is_equal, AOT.mult)
                            for e, (col_lo, _) in enumerate(epis):
                                nc.tensor.matmul(
                                    psTs[e][:],
                                    gbufs[b][:, rk, col_lo:col_lo + F],
                                    sT[:],
                                    start=(ji == 0), stop=(ji == nchunks - 1),
                                    skip_group_check=True)
                            ji += 1
                    for e, (_, epi_fn) in enumerate(epis):
                        epi_fn(t, psTs[e])
                if agp and si in agp:
                    for p in agp[si]:
                        agp["fire"](p)

        # epilogue: row-major hidden -> next gather table rows
        #   psT [fin, r] -> (A@X)@W + b (ones trick) -> prelu -> [r, h] bf16
        def epi_rowmajor(w_key, brow_key, dst, col_off):
            def epi(t, psT):
                R = rows_of(t)
                hT = hp.tile([128, 128], dt.bfloat16, tag="h")
                nc.scalar.activation(hT[:], psT[:], ACTF.Copy,
                                     bias=0.0, scale=1.0)
                ps = ps2.tile([128, F], dt.float32, tag="ps2")
                nc.tensor.matmul(ps[:R, :], hT[:, :R], wt[w_key][:],
                                 start=True, stop=False, skip_group_check=True)
                nc.tensor.matmul(ps[:R, :], ones[0:1, :R],
                                 brt[brow_key][0:1, :],
                                 start=False, stop=True, skip_group_check=True)
                h2 = hp.tile([128, F], dt.bfloat16, tag="h2")
                nc.scalar.activation(h2[:R, :], ps[:R, :], ACTF.Prelu,
                                     bias=bct["zcol"][:R, 0:1], scale=1.0,
                                     alpha=ALPHA)
                nc.sync.dma_start(
                    dst[t * 128:t * 128 + R, col_off:col_off + F], h2[:R, :])
            return epi

        # epilogue: final layer -> @W+b, prelu -> union linear -> output
        def epi_union(w_key, bcol_key, wt_key, wb_key, feaT, brow_key, out_t):
            def epi(t, psT):
                R = rows_of(t)
                hT = hp.tile([128, 128], dt.bfloat16, tag="h")
                nc.scalar.activation(hT[:], psT[:], ACTF.Copy,
                                     bias=0.0, scale=1.0)
                ps2T = ps2.tile([128, F], dt.float32, tag="ps2")
                nc.tensor.matmul(ps2T[:, :R], wt[w_key][:], hT[:, :R],
                                 start=True, stop=True, skip_group_check=True)
                u2T = hp.tile([128, 128], dt.bfloat16, tag="u2")
                nc.scalar.activation(u2T[:, :R], ps2T[:, :R], ACTF.Prelu,
                                     bias=bct[bcol_key][:, 0:1], scale=1.0,
                                     alpha=ALPHA)
                ft = fq.tile([128, 128], dt.bfloat16, tag="ft")
                nc.sync.dma_start(ft[:, :R], feaT[:, t * 128:t * 128 + R])
                psU = ps2.tile([128, F], dt.float32, tag="psU")
                nc.tensor.matmul(psU[:R, :], u2T[:, :R], wt[wt_key][:],
                                 start=True, stop=False, skip_group_check=True)
                nc.tensor.matmul(psU[:R, :], ft[:, :R], wt[wb_key][:],
                                 start=False, stop=False,
                                 skip_group_check=True)
                nc.tensor.matmul(psU[:R, :], ones[0:1, :R],
                                 brt[brow_key][0:1, :],
                                 start=False, stop=True, skip_group_check=True)
                osb = ob.tile([128, F], dt.float32, tag="o")
                nc.scalar.activation(osb[:R, :], psU[:R, :], ACTF.Relu,
                                     bias=0.0, scale=1.0)
                nc.sync.dma_start(out_t[t * 128:t * 128 + R, :], osb[:R, :])
            return epi

        # ---------------- stages
        agA = ag_plan(LVU)
        agA["fire"] = lambda p: ag_piece(tbc_sh, tbc_full, 2 * F, p)
        spmm("vu", lambda b: ufea_rows[b * BLK:(b + 1) * BLK, :], F,
             [(0, epi_rowmajor("W1", "b1_row", tbc_sh, F))],
             agp=agA)

        agBC = ag_plan(LUV)
        agBC["fire"] = lambda p: ag_piece(td_sh, td_full, F, p)
        spmm("uv", lambda b: tbc_full[b * BLK:(b + 1) * BLK, :], 2 * F,
             [(0, epi_rowmajor("W2", "b2_row", td_sh, 0)),
              (F, epi_union("W3", "b3", "Wut", "Wub", feaT_u, "bu_row",
                            out_user))],
             agp=agBC)

        spmm("vu", lambda b: td_full[b * BLK:(b + 1) * BLK, :], F,
             [(0, epi_union("W4", "b4", "Wit", "Wib", feaT_v, "bi_row",
                            out_item))])

    nc.compile()
    return nc, gather_insts


def _scheduled_queue_targets(nc, gather_insts):
    """Desired queue per gather issue-ordinal: the tile sem assigner rotates
    Pool DMA instructions over 8 DMASW lanes in SCHEDULED order; a lane must
    only ever see one SWDGE queue, so queue must equal (sched position % 4)."""
    import concourse.mybir as mybir
    from concourse.tile_scheduler import DMAInst
    from concourse import bass_isa

    name_to_ord = {inst.ins.name: i for i, inst in enumerate(gather_insts)}
    desired = [0] * len(gather_insts)
    pure = True
    p = 0
    for blk in nc.m.functions[0].blocks:
        for inst in blk.instructions:
            if inst.engine != mybir.EngineType.Pool:
                continue
            if isinstance(inst, DMAInst) and not isinstance(
                    inst, bass_isa.UserSyncedRemoteDMADescs):
                lane_q = p % 4
                o = name_to_ord.get(inst.name)
                if o is not None:
                    desired[o] = lane_q
                    if inst.queue_num != lane_q:
                        pure = False
                elif lane_q != 0:
                    pure = False
                p += 1
    return desired, pure


def _build_fixpoint(LVU, LUV, max_passes=3):
    qmap = None
    for _ in range(max_passes):
        nc, ginsts = _build(LVU, LUV, qmap)
        desired, pure = _scheduled_queue_targets(nc, ginsts)
        if pure:
            return nc
        qmap = desired
    nc, _ = _build(LVU, LUV, [0] * len(qmap))
    return nc


# ------------------------------------------------------------------ driver

def _run_and_time(nc, in_maps, iters=2):
    import os
    import time
    import jax
    from jax.sharding import Mesh, PartitionSpec, NamedSharding
    from jax.experimental.shard_map import shard_map
    from concourse import bass2jax
    import concourse.mybir as mybir

    bass2jax.install_neuronx_cc_hook()
    part_name = nc.partition_id_tensor.name if nc.partition_id_tensor else None
    in_names, out_names, out_avals, zero_outs = [], [], [], []
    for alloc in nc.m.functions[0].allocations:
        if not isinstance(alloc, mybir.MemoryLocationSet):
            continue
        name = alloc.memorylocations[0].name
        if alloc.kind == "ExternalInput":
            if name != part_name:
                in_names.append(name)
        elif alloc.kind == "ExternalOutput":
            out_names.append(name)
            shape = tuple(alloc.tensor_shape)
            dty = mybir.dt.np(alloc.dtype)
            out_avals.append(jax.core.ShapedArray(shape, dty))
            zero_outs.append(np.zeros(shape, dty))
    n_params = len(in_names)
    all_in = list(in_names) + list(out_names)
    if part_name:
        all_in.append(part_name)

    def _body(*args):
        operands = list(args)
        if part_name:
            operands.append(bass2jax.partition_id_tensor())
        outs = bass2jax._bass_exec_p.bind(
            *operands, out_avals=tuple(out_avals), in_names=tuple(all_in),
            out_names=tuple(out_names), lowering_input_output_aliases=(),
            sim_require_finite=True, sim_require_nnan=True, nc=nc)
        return tuple(outs)

    devices = jax.devices()[:NC]
    mesh = Mesh(np.asarray(devices), ("core",))
    nio = n_params + len(out_names)
    sharded = jax.jit(
        shard_map(_body, mesh=mesh, in_specs=(PartitionSpec("core"),) * nio,
                  out_specs=(PartitionSpec("core"),) * len(out_names),
                  check_rep=False),
        keep_unused=True)

    sh = NamedSharding(mesh, PartitionSpec("core"))
    dev_in = [jax.device_put(
        np.concatenate([np.asarray(m[name]) for m in in_maps], 0), sh)
        for name in in_names]
    dev_zero = [jax.device_put(
        np.zeros((NC * z.shape[0], *z.shape[1:]), z.dtype), sh)
        for z in zero_outs]

    out = sharded(*dev_in, *dev_zero)
    jax.block_until_ready(out)
    results = [
        {name: np.asarray(out[i]).reshape(NC, *out_avals[i].shape)[c]
         for i, name in enumerate(out_names)}
        for c in range(NC)]

    npipe = int(os.environ.get("KERNEL_PIPE_N", "64"))
    best = None
    for _ in range(iters):
        jax.block_until_ready(sharded(*dev_in, *dev_zero))
        t0 = time.perf_counter()
        outs = [sharded(*dev_in, *dev_zero) for _ in range(npipe)]
        jax.block_until_ready(outs)
        dtns = (time.perf_counter() - t0) * 1e9 / npipe
        best = dtns if best is None else min(best, dtns)
    return results, best


def _prep_inputs(inputs):
    ufea = np.asarray(inputs["ufea"], np.float32)
    vfea = np.asarray(inputs["vfea"], np.float32)

    LVU, vu_cores = _prep_adj(inputs["vu_rows"], inputs["vu_cols"],
                              inputs["vu_vals"], ST_A)
    LUV, uv_cores = _prep_adj(inputs["uv_rows"], inputs["uv_cols"],
                              inputs["uv_vals"], ST_BC)

    W = {k: np.asarray(inputs[k], np.float32) for k in
         ["W1", "b1", "W2", "b2", "W3", "b3", "W4", "b4",
          "Wu", "bu", "Wi", "bi"]}
    common = dict(
        W1=W["W1"].astype(BF16), W2=W["W2"].astype(BF16),
        W3=W["W3"].astype(BF16), W4=W["W4"].astype(BF16),
        Wut=np.ascontiguousarray(W["Wu"][:F]).astype(BF16),
        Wub=np.ascontiguousarray(W["Wu"][F:]).astype(BF16),
        Wit=np.ascontiguousarray(W["Wi"][:F]).astype(BF16),
        Wib=np.ascontiguousarray(W["Wi"][F:]).astype(BF16),
        b1_row=W["b1"].reshape(1, F).astype(BF16),
        b2_row=W["b2"].reshape(1, F).astype(BF16),
        b3=W["b3"].reshape(F, 1), b4=W["b4"].reshape(F, 1),
        zcol=np.zeros((F, 1), np.float32),
        bu_row=W["bu"].reshape(1, F).astype(BF16),
        bi_row=W["bi"].reshape(1, F).astype(BF16),
        iota=np.tile(np.arange(128).astype(BF16)[None, :], (128, 1)),
        ones=np.ones((1, 128), BF16),
    )
    ufea_perm = np.empty((N, F), BF16)
    ufea_perm[_perm(np.arange(N))] = ufea.astype(BF16)
    common["ufea_rows"] = ufea_perm

    ufeaT_full = np.ascontiguousarray(ufea.T).astype(BF16)
    vfeaT_full = np.ascontiguousarray(vfea.T).astype(BF16)
    vfea_bf = np.ascontiguousarray(vfea).astype(BF16)
    in_maps = []
    for c in range(NC):
        m = dict(common)
        m["vfea_sh"] = np.ascontiguousarray(vfea_bf[c * RPC:(c + 1) * RPC])
        m["feaT_u"] = np.ascontiguousarray(ufeaT_full[:, c * RPC:(c + 1) * RPC])
        m["feaT_v"] = np.ascontiguousarray(vfeaT_full[:, c * RPC:(c + 1) * RPC])
        for tag, cores in (("vu", vu_cores), ("uv", uv_cores)):
            m[f"idx_{tag}"] = cores[c]["idx"]
            m[f"rr_{tag}"] = cores[c]["rr"]
            m[f"vv_{tag}"] = cores[c]["vv"]
        in_maps.append(m)
    return LVU, LUV, in_maps


def kernel(**inputs):
    global LAST_EXEC_NS

    LVU, LUV, in_maps = _prep_inputs(inputs)
    nc = _build_fixpoint(LVU, LUV)

    results, wall_ns = _run_and_time(
        nc, in_maps,
        iters=int(__import__("os").environ.get("KERNEL_BENCH_ITERS", "3")))
    LAST_EXEC_NS = int(wall_ns)

    user = np.concatenate([results[c]["user_out"] for c in range(NC)], 0)
    item = np.concatenate([results[c]["item_out"] for c in range(NC)], 0)
    return (user, item)
